# revision 1
# baseline (speedup 1.0000x reference)
"""Trainium2 Bass kernel for 50-iteration Jacobi (3x3 cross stencil, reflect pad).

x_{t+1} = 0.25*(V + H) x_t + f,  f = COF*layout (|f| < 2.4e-9 -- numerically
negligible vs |x| ~ 0.1, contributes < 3e-6 relative to the output; dropped).

Strategy per core (2 of 16 images, all state resident in SBUF):
  - k-step fusion: x_{t+k} = sum_j 0.25^k C(k,j) V^{k-j} (H^j x),  V/H commute.
  - H^j chain: DVE shifted adds along the free dim (+ reflect edge fixes).
  - V^{k-j} terms: TensorE fp32r matmuls with block-banded 128x128 weights
    (exact: small ints x 2^-6), accumulated in PSUM.
  - combine: scalar_tensor_tensor  x_new = 0.25^k * H^k x + PSUM.
Image rows tiled 8 x [128 part, 1024 cols]; stored as [128, 8192] SBUF bufs.
"""

import math
from contextlib import ExitStack

import numpy as np

NX = 1024
NT = 8  # row tiles per image
IMGS_PER_CORE = 2
N_CORES = 8
KMAX = 3

_compiled_cache = {}


def _vertical_matrix():
    A = np.zeros((NX, NX), np.float64)
    for i in range(NX):
        A[i, i - 1 if i > 0 else 1] += 1.0
        A[i, i + 1 if i < NX - 1 else NX - 2] += 1.0
    return A


def _plan_steps(n_iter):
    q, r = divmod(n_iter, KMAX)
    return [KMAX] * q + ([r] if r else [])


def _build_blocks(ks_needed):
    """Unique lhsT 128x128 blocks for every (k, j, diag, out_tile)."""
    A = _vertical_matrix()
    pows = {0: np.eye(NX)}
    for p in range(1, KMAX + 1):
        pows[p] = pows[p - 1] @ A
    uniq = {}
    blocks = []
    bmap = {}
    for k in sorted(set(ks_needed)):
        for j in range(k + 1):
            # j == k is the identity term (H^k coefficient), used when the
            # combine runs as identity-matmul + ACT copy instead of DVE stt.
            Op = (0.25 ** k * math.comb(k, j)) * pows[k - j]
            for og in range(NT):
                for d in (-1, 0, 1):
                    sg = og + d
                    if not 0 <= sg < NT:
                        continue
                    if j == k and d != 0:
                        continue
                    blk = np.ascontiguousarray(
                        Op[og * 128:(og + 1) * 128, sg * 128:(sg + 1) * 128].T
                    ).astype(np.float32)
                    key = blk.tobytes()
                    if key not in uniq:
                        uniq[key] = len(blocks)
                        blocks.append(blk)
                    bmap[(k, j, d, og)] = uniq[key]
    return np.stack(blocks), bmap


def _build_program(n_iter):
    import concourse.bacc as bacc
    import concourse.mybir as mybir
    import concourse.tile as tile

    steps = _plan_steps(n_iter)
    wb_np, bmap = _build_blocks(steps)
    nu = wb_np.shape[0]
    f32r = mybir.dt.float32r
    f32 = mybir.dt.float32
    add = mybir.AluOpType.add
    mult = mybir.AluOpType.mult

    nc = bacc.Bacc("TRN2", target_bir_lowering=False, debug=False)
    x0_d = nc.dram_tensor("x0", [IMGS_PER_CORE * NX, NX], f32r,
                          kind="ExternalInput").ap()
    wb_d = nc.dram_tensor("wb", [nu, 128, 128], f32r, kind="ExternalInput").ap()
    y_d = nc.dram_tensor("y", [IMGS_PER_CORE * NX, NX], f32,
                         kind="ExternalOutput").ap()

    with tile.TileContext(nc) as tc, ExitStack() as ctx:
        wp = ctx.enter_context(tc.tile_pool(name="w", bufs=1))
        bp = ctx.enter_context(tc.tile_pool(name="b", bufs=1))
        pp = ctx.enter_context(tc.tile_pool(name="ps", bufs=4, space="PSUM"))

        wt = wp.tile([128, nu * 128], f32r)
        for u in range(nu):
            nc.sync.dma_start(wt[:, u * 128:(u + 1) * 128], wb_d[u, :, :])

        xa = bp.tile([128, NT * NX], f32r, tag="xa")
        xb = bp.tile([128, NT * NX], f32r, tag="xb")
        hs = [bp.tile([128, NT * NX], f32r, name=f"h{j}", tag=f"h{j}")
              for j in range(KMAX)]

        W = NT * NX  # 8192
        ACT_TILES = (0, 1, 2, 3)  # combine via identity-matmul + ACT copy
        HALVES = ((0, 4), (4, 8))  # h-pass block ranges

        def happly(dst, src, b0, b1):
            """dst = H(src) for blocks [b0,b1): shifted add + reflect fixes."""
            lo, hi = b0 * NX, b1 * NX
            nc.vector.tensor_tensor(
                dst[:, lo + 1:hi - 1], src[:, lo:hi - 2].bitcast(f32),
                src[:, lo + 2:hi].bitcast(f32), op=add)
            d3 = dst[:].rearrange("p (g c) -> p g c", c=NX)
            s3 = src[:].rearrange("p (g c) -> p g c", c=NX)
            nc.scalar.mul(d3[:, b0:b1, 0:1], s3[:, b0:b1, 1:2].bitcast(f32), 2.0)
            nc.scalar.mul(d3[:, b0:b1, NX - 1:NX],
                          s3[:, b0:b1, NX - 2:NX - 1].bitcast(f32), 2.0)

        def step(k, xc, xn):
            # DVE h-chain in halves (H is 1024-block independent)
            prev = xc
            for j in range(k):
                for b0, b1 in HALVES:
                    happly(hs[j], prev, b0, b1)
                prev = hs[j]
            for grp in (range(0, 4), range(4, 8)):
                Ps = {}
                mms = {}
                for og in grp:
                    Ps[og] = pp.tile([128, NX], f32, name=f"P{og}", tag="ps")
                    for hf in range(2):
                        lst = []
                        for j in range(k):
                            rhs = xc if j == 0 else hs[j - 1]
                            for d in (-1, 0, 1):
                                sg = og + d
                                if 0 <= sg < NT:
                                    lst.append((j, bmap[(k, j, d, og)], rhs, sg))
                        if og in ACT_TILES:
                            lst.append((k, bmap[(k, k, 0, og)], hs[k - 1], og))
                        mms[(og, hf)] = lst
                # j-major emission keeps the in-order PE queue unblocked
                nlev = max(len(v) for v in mms.values())
                for lev in range(nlev):
                    for og in grp:
                        for hf in range(2):
                            lst = mms[(og, hf)]
                            if lev >= len(lst):
                                continue
                            j, u, rhs, sg = lst[lev]
                            dst = Ps[og][:, hf * 512:hf * 512 + 512]
                            nc.tensor.matmul(
                                dst, wt[:, u * 128:(u + 1) * 128],
                                rhs[:, sg * NX + hf * 512: sg * NX + hf * 512 + 512],
                                start=(lev == 0), stop=(lev == len(lst) - 1))
                for og in grp:
                    if og in ACT_TILES:
                        nc.scalar.copy(xn[:, og * NX:(og + 1) * NX], Ps[og][:])
                    else:
                        nc.vector.scalar_tensor_tensor(
                            xn[:, og * NX:(og + 1) * NX],
                            hs[k - 1][:, og * NX:(og + 1) * NX].bitcast(f32),
                            0.25 ** k, Ps[og][:], op0=mult, op1=add)

        for img in range(IMGS_PER_CORE):
            r0 = img * NX
            for g in range(NT):
                nc.sync.dma_start(xa[:, g * NX:(g + 1) * NX],
                                  x0_d[r0 + g * 128: r0 + (g + 1) * 128, :])
            cur, nxt = xa, xb
            for k in steps:
                step(k, cur, nxt)
                cur, nxt = nxt, cur
            for g in range(NT):
                nc.sync.dma_start(y_d[r0 + g * 128: r0 + (g + 1) * 128, :],
                                  cur[:, g * NX:(g + 1) * NX].bitcast(f32))

    nc.compile()
    return nc, wb_np


def kernel(layout, heat, n_iter):
    n_iter = int(n_iter)
    heat = np.asarray(heat, dtype=np.float32)
    out_shape = heat.shape
    x = heat.reshape(16, NX, NX)
    if n_iter <= 0:
        return heat.copy()

    from concourse.bass_utils import run_bass_kernel_spmd

    if n_iter not in _compiled_cache:
        _compiled_cache[n_iter] = _build_program(n_iter)
    nc, wb_np = _compiled_cache[n_iter]

    in_maps = []
    for c in range(N_CORES):
        shard = np.ascontiguousarray(
            x[c * IMGS_PER_CORE:(c + 1) * IMGS_PER_CORE].reshape(
                IMGS_PER_CORE * NX, NX))
        in_maps.append({"x0": shard, "wb": wb_np})
    res = run_bass_kernel_spmd(nc, in_maps, core_ids=list(range(N_CORES)))
    out = np.empty((16, NX, NX), np.float32)
    for c in range(N_CORES):
        out[c * IMGS_PER_CORE:(c + 1) * IMGS_PER_CORE] = (
            res.results[c]["y"].reshape(IMGS_PER_CORE, NX, NX))
    return out.reshape(out_shape)



# revision 3
# speedup vs baseline: 9.3867x; 9.3867x over previous
"""Trainium2 Bass kernel for n-iteration Jacobi (3x3 cross stencil, reflect pad).

Spectral method: the iteration x <- 0.25*(V+H)x (+ f, dropped: |f| < 2.4e-9
contributes < 3e-7 relative) is exactly diagonalized by the DCT-I basis
v_k[i] = cos(pi*k*i/(NX-1)) with eigenvalues lam_kl = (cos(pi k/M)+cos(pi l/M))/2.

  x_n = C^T [ Lam^n  o  (Cw x Cw^T) ] C      (o = elementwise)

lam^n kills all but the lowest and highest (checkerboard) frequency strips, so
only K = 512 of 1024 modes per axis are kept (rel truncation err ~2e-4 for
n=50, verified on host).  Four dense matmul passes per image, ZERO transposes:
alternate which operand is stationary (the data X / Z' go in as lhsT).

  A: YT[c,k] = sum_i  X[i,c]   * CTw[i,k]     (lhsT = X blocks)
  B: Z [l,k] = sum_c  CTw[c,l] * YT[c,k]      (lhsT = CTw)   -> DVE *lam table
  C: WT[k,c] = sum_l  Zp[l,k]  * Csel[l,c]    (lhsT = Zp)
  D: XN[i,c] = sum_k  Csel[k,i]* WT[k,c]      (lhsT = Csel)

Per core: 2 of 16 images (data parallel over batch), everything SBUF-resident.
"""

import numpy as np
from contextlib import ExitStack

NX = 1024
M = NX - 1
NB = 8           # 128-row blocks per image
IMGS_PER_CORE = 2
N_CORES = 8

_compiled_cache = {}


def _pick_nk(n_iter):
    """Smallest strip width nk (K=2*nk kept modes/axis) with safe truncation.

    Error estimate: dropped-mode field has pointwise std
    ~ sqrt(sum_dropped lam^(2n) / (n_k n_l)); require 6 sigma < 1e-3 * 0.4.
    """
    lam1 = 0.5 * np.cos(np.pi * np.arange(NX) / M)
    nrm = np.full(NX, M / 2.0)
    nrm[0] = nrm[-1] = float(M)
    lam2 = np.abs(lam1[:, None] + lam1[None, :]) ** (2 * n_iter) / np.outer(nrm, nrm)
    for nk in (192, 256, 320, 384):
        keepmask = np.zeros(NX, bool)
        keepmask[:nk] = True
        keepmask[NX - nk:] = True
        drop = lam2 * ~(keepmask[:, None] & keepmask[None, :])
        if 6.0 * np.sqrt(drop.sum()) < 4e-4:
            return nk
    return None  # n too small for truncation with K<=768 -> host fallback


def _build_tables(n_iter, nk):
    keep = np.r_[0:nk, NX - nk:NX]
    K = keep.size
    KB = K // 128
    idx = np.arange(NX)
    C = np.cos(np.pi * np.outer(keep, idx) / M)        # [K, NX]
    w = np.ones(NX)
    w[0] = w[-1] = 0.5
    nrm = np.full(NX, M / 2.0)
    nrm[0] = nrm[-1] = float(M)
    lam = 0.5 * np.cos(np.pi * keep / M)
    lam2 = (lam[:, None] + lam[None, :]) ** n_iter / np.outer(nrm[keep], nrm[keep])
    CTw = (C * w[None, :]).T                           # [NX, K]
    ctw_np = np.ascontiguousarray(CTw.reshape(NB, 128, K)).astype(np.float32)
    csel_np = np.ascontiguousarray(C.reshape(KB, 128, NX)).astype(np.float32)
    lamt_np = np.ascontiguousarray(lam2.reshape(KB, 128, K)).astype(np.float32)
    return ctw_np, csel_np, lamt_np


def _build_program(n_iter, nk):
    import concourse.bacc as bacc
    import concourse.mybir as mybir
    import concourse.tile as tile

    ctw_np, csel_np, lamt_np = _build_tables(n_iter, nk)
    K = 2 * nk
    KB = K // 128
    KH = K // 512            # 512-wide k-halves (1 for K=512)
    f32r = mybir.dt.float32r
    f32 = mybir.dt.float32
    mult = mybir.AluOpType.mult

    nc = bacc.Bacc("TRN2", target_bir_lowering=False, debug=False)
    x0_d = nc.dram_tensor("x0", [IMGS_PER_CORE * NX, NX], f32r,
                          kind="ExternalInput").ap()
    ctw_d = nc.dram_tensor("ctw", [NB, 128, K], f32r, kind="ExternalInput").ap()
    csel_d = nc.dram_tensor("csel", [KB, 128, NX], f32r,
                            kind="ExternalInput").ap()
    lamt_d = nc.dram_tensor("lamt", [KB, 128, K], f32,
                            kind="ExternalInput").ap()
    y_d = nc.dram_tensor("y", [IMGS_PER_CORE * NX, NX], f32,
                         kind="ExternalOutput").ap()

    with tile.TileContext(nc) as tc, ExitStack() as ctx:
        wp = ctx.enter_context(tc.tile_pool(name="w", bufs=1))
        bp = ctx.enter_context(tc.tile_pool(name="b", bufs=1))
        pp = ctx.enter_context(tc.tile_pool(name="ps", bufs=8, space="PSUM"))

        ctw_t = wp.tile([128, NB * K], f32r, name="ctw", tag="ctw")
        csel_t = wp.tile([128, KB * NX], f32r, name="csel", tag="csel")
        lamt_t = wp.tile([128, KB * K], f32, name="lamt", tag="lamt")
        for b in range(NB):
            nc.sync.dma_start(ctw_t[:, b * K:(b + 1) * K], ctw_d[b])
        for m in range(KB):
            nc.sync.dma_start(csel_t[:, m * NX:(m + 1) * NX], csel_d[m])
        for m in range(KB):
            nc.sync.dma_start(lamt_t[:, m * K:(m + 1) * K], lamt_d[m])

        Xs = [bp.tile([128, NB * NX], f32r, name=f"x{i}", tag=f"x{i}")
              for i in range(IMGS_PER_CORE)]
        YT = bp.tile([128, NB * K], f32r, name="yt", tag="yt")
        ZP = bp.tile([128, KB * K], f32r, name="zp", tag="zp")
        WT = bp.tile([128, KB * NX], f32r, name="wt", tag="wt")

        # hoist all input DMAs: image 1 streams in during image 0 compute
        for img in range(IMGS_PER_CORE):
            for b in range(NB):
                nc.sync.dma_start(Xs[img][:, b * NX:(b + 1) * NX],
                                  x0_d[img * NX + b * 128:
                                       img * NX + (b + 1) * 128, :])

        def evict(dst_ap, src_tile, idx):
            # alternate engines to halve eviction latency
            if idx % 2 == 0:
                nc.scalar.copy(dst_ap, src_tile[:])
            else:
                nc.vector.tensor_copy(dst_ap, src_tile[:])

        for img in range(IMGS_PER_CORE):
            X = Xs[img]

            # ---- pass A: YT[c-blk g, k-half kh] += X(b,g)^T @ CTw[b, kh]
            # chunk-major so matmuls start as soon as X chunk b lands
            tiles_a = [(g, kh) for g in range(NB) for kh in range(KH)]
            for wave0 in range(0, len(tiles_a), 8):
                wave = tiles_a[wave0:wave0 + 8]
                Pa = {t: pp.tile([128, 512], f32, name=f"A{img}_{t[0]}_{t[1]}",
                                 tag="ps") for t in wave}
                for b in range(NB):
                    for (g, kh) in wave:
                        nc.tensor.matmul(
                            Pa[(g, kh)][:],
                            X[:, b * NX + g * 128: b * NX + (g + 1) * 128],
                            ctw_t[:, b * K + kh * 512: b * K + kh * 512 + 512],
                            start=(b == 0), stop=(b == NB - 1))
                for i, (g, kh) in enumerate(wave):
                    evict(YT[:, g * K + kh * 512: g * K + kh * 512 + 512],
                          Pa[(g, kh)], i)

            # ---- pass B: Z[l-blk m, kh] += CTw(b,m)^T @ YT[b, kh]; DVE *lam
            tiles_b = [(m, kh) for m in range(KB) for kh in range(KH)]
            for wave0 in range(0, len(tiles_b), 8):
                wave = tiles_b[wave0:wave0 + 8]
                Pb = {t: pp.tile([128, 512], f32, name=f"B{img}_{t[0]}_{t[1]}",
                                 tag="ps") for t in wave}
                for (m, kh) in wave:
                    for b in range(NB):
                        nc.tensor.matmul(
                            Pb[(m, kh)][:],
                            ctw_t[:, b * K + m * 128: b * K + (m + 1) * 128],
                            YT[:, b * K + kh * 512: b * K + kh * 512 + 512],
                            start=(b == 0), stop=(b == NB - 1))
                    nc.vector.tensor_tensor(
                        ZP[:, m * K + kh * 512: m * K + kh * 512 + 512],
                        lamt_t[:, m * K + kh * 512: m * K + kh * 512 + 512],
                        Pb[(m, kh)][:], op=mult)

            # ---- pass C: WT[k-blk m, c-half hf] += Zp(l,m)^T @ Csel[l, hf]
            tiles_c = [(m, hf) for m in range(KB) for hf in range(2)]
            for wave0 in range(0, len(tiles_c), 8):
                wave = tiles_c[wave0:wave0 + 8]
                Pc = {t: pp.tile([128, 512], f32, name=f"C{img}_{t[0]}_{t[1]}",
                                 tag="ps") for t in wave}
                for (m, hf) in wave:
                    for l in range(KB):
                        nc.tensor.matmul(
                            Pc[(m, hf)][:],
                            ZP[:, l * K + m * 128: l * K + (m + 1) * 128],
                            csel_t[:, l * NX + hf * 512: l * NX + hf * 512 + 512],
                            start=(l == 0), stop=(l == KB - 1))
                for i, (m, hf) in enumerate(wave):
                    evict(WT[:, m * NX + hf * 512: m * NX + hf * 512 + 512],
                          Pc[(m, hf)], i)

            # ---- pass D: XN[og g, c-half hf] += Csel(m,g)^T @ WT[m, hf]
            # output overwrites X (consumed by pass A); DMA out per og tile
            tiles_d = [(g, hf) for g in range(NB) for hf in range(2)]
            for wave0 in range(0, len(tiles_d), 8):
                wave = tiles_d[wave0:wave0 + 8]
                Pd = {t: pp.tile([128, 512], f32, name=f"D{img}_{t[0]}_{t[1]}",
                                 tag="ps") for t in wave}
                for (g, hf) in wave:
                    for m in range(KB):
                        nc.tensor.matmul(
                            Pd[(g, hf)][:],
                            csel_t[:, m * NX + g * 128: m * NX + (g + 1) * 128],
                            WT[:, m * NX + hf * 512: m * NX + hf * 512 + 512],
                            start=(m == 0), stop=(m == KB - 1))
                for i, (g, hf) in enumerate(wave):
                    evict(X[:, g * NX + hf * 512: g * NX + hf * 512 + 512],
                          Pd[(g, hf)], i)
                done = {g for g in range(NB)
                        if (g, 0) in wave and (g, 1) in wave}
                for g in sorted(done):
                    nc.sync.dma_start(
                        y_d[img * NX + g * 128: img * NX + (g + 1) * 128, :],
                        X[:, g * NX:(g + 1) * NX].bitcast(f32))

    nc.compile()
    return nc, ctw_np, csel_np, lamt_np


def _host_reference(heat, n_iter):
    x = heat.reshape(16, NX, NX).astype(np.float32).copy()
    xp = np.empty((16, NX + 2, NX + 2), np.float32)
    for _ in range(n_iter):
        xp[:, 1:-1, 1:-1] = x
        xp[:, 0, 1:-1] = x[:, 1]
        xp[:, -1, 1:-1] = x[:, -2]
        xp[:, 1:-1, 0] = x[:, :, 1]
        xp[:, 1:-1, -1] = x[:, :, -2]
        x = 0.25 * (xp[:, :-2, 1:-1] + xp[:, 2:, 1:-1]
                    + xp[:, 1:-1, :-2] + xp[:, 1:-1, 2:])
    return x


def kernel(layout, heat, n_iter):
    n_iter = int(n_iter)
    heat = np.asarray(heat, dtype=np.float32)
    out_shape = heat.shape
    if n_iter <= 0:
        return heat.copy()

    nk = _pick_nk(n_iter)
    if nk is None:
        # n too small for safe spectral truncation (never hit for n=50);
        # exact host iteration keeps the kernel correct for any n_iter.
        return _host_reference(heat, n_iter).reshape(out_shape)

    from concourse.bass_utils import run_bass_kernel_spmd

    key = (n_iter, nk)
    if key not in _compiled_cache:
        _compiled_cache[key] = _build_program(n_iter, nk)
    nc, ctw_np, csel_np, lamt_np = _compiled_cache[key]

    x = heat.reshape(16, NX, NX)
    in_maps = []
    for c in range(N_CORES):
        shard = np.ascontiguousarray(
            x[c * IMGS_PER_CORE:(c + 1) * IMGS_PER_CORE].reshape(
                IMGS_PER_CORE * NX, NX))
        in_maps.append({"x0": shard, "ctw": ctw_np, "csel": csel_np,
                        "lamt": lamt_np})
    res = run_bass_kernel_spmd(nc, in_maps, core_ids=list(range(N_CORES)))
    out = np.empty((16, NX, NX), np.float32)
    for c in range(N_CORES):
        out[c * IMGS_PER_CORE:(c + 1) * IMGS_PER_CORE] = (
            res.results[c]["y"].reshape(IMGS_PER_CORE, NX, NX))
    return out.reshape(out_shape)


# revision 4
# speedup vs baseline: 11.7327x; 1.2499x over previous
"""Trainium2 Bass kernel for n-iteration Jacobi (3x3 cross stencil, reflect pad).

Spectral method: the iteration x <- 0.25*(V+H)x (+ f, dropped: |f| < 2.4e-9
contributes < 3e-7 relative) is exactly diagonalized by the DCT-I basis
v_k[i] = cos(pi*k*i/M), M = NX-1, eigenvalues lam_kl = (cos(pi k/M)+cos(pi l/M))/2.

  x_n = C^T [ Lam^n o (Cw x Cw^T) ] C        (o = elementwise)

lam^n kills all but the lowest and highest (checkerboard) frequency strips:
only K = 512 of 1024 modes per axis are kept (truncation err ~2e-4 for n=50,
verified on host), and of the KxK coefficient block only the (low,low) and
(high,high) boxes survive (cross boxes have |lam| <= 0.15 -> lam^50 ~ 1e-43),
halving passes B/C.  Four dense matmul passes per image, ZERO transposes:
alternate which operand is stationary (the data X / Z' go in as lhsT).

  A: YT[c,k] = sum_i  X[i,c]   * CTw[i,k]     (lhsT = X blocks)
  B: Z [l,k] = sum_c  CTw[c,l] * YT[c,k]      (lhsT = CTw)   -> DVE *lam table
  C: WT[k,c] = sum_l  Zp[l,k]  * Csel[l,c]    (lhsT = Zp)
  D: XN[i,c] = sum_k  Csel[k,i]* WT[k,c]      (lhsT = Csel)

Per core: 2 of 16 images (data parallel over batch), everything SBUF-resident.
DMA dispatch is spread over the two HWDGE engines (sync + scalar): pass-A
inputs (ctw + X img0) are dispatched first/interleaved so matmuls start ~3us
in; all other loads are deferred behind pass A's emission.  Output tiles are
written back in row-halves on alternating engines to shorten the tail.
"""

import numpy as np
from contextlib import ExitStack

NX = 1024
M = NX - 1
NB = 8           # 128-row blocks per image
IMGS_PER_CORE = 2
N_CORES = 8

_compiled_cache = {}


def _pick_nk(n_iter):
    """Smallest strip width nk (K=2*nk kept modes/axis) with safe truncation.

    Dropped-mode field pointwise std ~ sqrt(sum_dropped lam^(2n)/(n_k n_l));
    require 6 sigma < 4e-4 (abs, vs |x|_max ~ 0.5 -> ~1e-3 relative budget).
    """
    lam1 = 0.5 * np.cos(np.pi * np.arange(NX) / M)
    nrm = np.full(NX, M / 2.0)
    nrm[0] = nrm[-1] = float(M)
    lam2 = np.abs(lam1[:, None] + lam1[None, :]) ** (2 * n_iter) / np.outer(nrm, nrm)
    for nk in (192, 256, 320, 384):
        keepmask = np.zeros(NX, bool)
        keepmask[:nk] = True
        keepmask[NX - nk:] = True
        drop = lam2 * ~(keepmask[:, None] & keepmask[None, :])
        if 6.0 * np.sqrt(drop.sum()) < 4e-4:
            return nk
    return None  # n too small for truncation with K<=768 -> host fallback


def _build_tables(n_iter, nk):
    keep = np.r_[0:nk, NX - nk:NX]
    K = keep.size
    KB = K // 128
    idx = np.arange(NX)
    C = np.cos(np.pi * np.outer(keep, idx) / M)        # [K, NX]
    w = np.ones(NX)
    w[0] = w[-1] = 0.5
    nrm = np.full(NX, M / 2.0)
    nrm[0] = nrm[-1] = float(M)
    lam = 0.5 * np.cos(np.pi * keep / M)
    lam2 = (lam[:, None] + lam[None, :]) ** n_iter / np.outer(nrm[keep], nrm[keep])
    CTw = (C * w[None, :]).T                           # [NX, K]
    ctw_np = np.ascontiguousarray(CTw.reshape(NB, 128, K)).astype(np.float32)
    csel_np = np.ascontiguousarray(C.reshape(KB, 128, NX)).astype(np.float32)
    lamt_np = np.ascontiguousarray(lam2.reshape(KB, 128, K)).astype(np.float32)
    return ctw_np, csel_np, lamt_np


def _build_program(n_iter, nk):
    import concourse.bacc as bacc
    import concourse.mybir as mybir
    import concourse.tile as tile

    ctw_np, csel_np, lamt_np = _build_tables(n_iter, nk)
    K = 2 * nk
    KB = K // 128
    KH = K // 2              # half-width of the kept-mode strip
    NL = KB // 2             # number of 128-blocks in the low strip
    # only skip dead cross boxes when halves are block-aligned and the
    # reduced moving dim keeps fp32r at full rate
    boxskip = (KB % 2 == 0) and (KH >= 256)
    f32r = mybir.dt.float32r
    f32 = mybir.dt.float32
    mult = mybir.AluOpType.mult

    nc = bacc.Bacc("TRN2", target_bir_lowering=False, debug=False)
    x0_d = nc.dram_tensor("x0", [IMGS_PER_CORE * NX, NX], f32r,
                          kind="ExternalInput").ap()
    ctw_d = nc.dram_tensor("ctw", [NB, 128, K], f32r, kind="ExternalInput").ap()
    csel_d = nc.dram_tensor("csel", [KB, 128, NX], f32r,
                            kind="ExternalInput").ap()
    lamt_d = nc.dram_tensor("lamt", [KB, 128, K], f32,
                            kind="ExternalInput").ap()
    y_d = nc.dram_tensor("y", [IMGS_PER_CORE * NX, NX], f32,
                         kind="ExternalOutput").ap()

    with tile.TileContext(nc) as tc, ExitStack() as ctx:
        wp = ctx.enter_context(tc.tile_pool(name="w", bufs=1))
        bp = ctx.enter_context(tc.tile_pool(name="b", bufs=1))
        pp = ctx.enter_context(tc.tile_pool(name="ps", bufs=8, space="PSUM"))

        ctw_t = wp.tile([128, NB * K], f32r, name="ctw", tag="ctw")
        csel_t = wp.tile([128, KB * NX], f32r, name="csel", tag="csel")
        lamt_t = wp.tile([128, KB * K], f32, name="lamt", tag="lamt")

        Xs = [bp.tile([128, NB * NX], f32r, name=f"x{i}", tag=f"x{i}")
              for i in range(IMGS_PER_CORE)]
        YT = bp.tile([128, NB * K], f32r, name="yt", tag="yt")
        ZP = bp.tile([128, KB * K], f32r, name="zp", tag="zp")
        WT = bp.tile([128, KB * NX], f32r, name="wt", tag="wt")

        # pass-A-critical loads, interleaved across both HWDGE engines so the
        # first chunk's inputs land within a few us
        for b in range(NB):
            nc.scalar.dma_start(ctw_t[:, b * K:(b + 1) * K], ctw_d[b])
            nc.sync.dma_start(Xs[0][:, b * NX:(b + 1) * NX],
                              x0_d[b * 128:(b + 1) * 128, :])

        def evict(dst_ap, src_ap, idx):
            # alternate engines to halve eviction latency
            if idx % 2 == 0:
                nc.scalar.copy(dst_ap, src_ap)
            else:
                nc.vector.tensor_copy(dst_ap, src_ap)

        def pass_a(img):
            # YT[c-blk g, k-half kh] += X(b,g)^T @ CTw[b, kh], chunk-major so
            # matmuls start as soon as X chunk b lands
            X = Xs[img]
            tiles = [(g, kh) for g in range(NB) for kh in range(0, K, 512)]
            for w0 in range(0, len(tiles), 8):
                wave = tiles[w0:w0 + 8]
                P = {t: pp.tile([128, 512], f32, name=f"A{img}_{t[0]}_{t[1]}",
                                tag="ps") for t in wave}
                for b in range(NB):
                    for (g, kh) in wave:
                        nc.tensor.matmul(
                            P[(g, kh)][:],
                            X[:, b * NX + g * 128: b * NX + (g + 1) * 128],
                            ctw_t[:, b * K + kh: b * K + kh + 512],
                            start=(b == 0), stop=(b == NB - 1))
                for i, (g, kh) in enumerate(wave):
                    evict(YT[:, g * K + kh: g * K + kh + 512],
                          P[(g, kh)][:], i)

        def pass_b(img):
            # Z[l-blk m, k-cols] += CTw(b,m)^T @ YT[b, k-cols]; DVE *lam evict.
            # boxskip: low-strip blocks only need the low k-half (and high-high)
            for m in range(KB):
                if boxskip:
                    spans = [(0, KH) if m < NL else (KH, K)]
                else:
                    spans = [(kh, min(kh + 512, K)) for kh in range(0, K, 512)]
                for (k0, k1) in spans:
                    P = pp.tile([128, 512], f32, name=f"B{img}_{m}_{k0}",
                                tag="ps")
                    kw = k1 - k0
                    for b in range(NB):
                        nc.tensor.matmul(
                            P[:, 0:kw],
                            ctw_t[:, b * K + m * 128: b * K + (m + 1) * 128],
                            YT[:, b * K + k0: b * K + k1],
                            start=(b == 0), stop=(b == NB - 1))
                    nc.vector.tensor_tensor(
                        ZP[:, m * K + k0: m * K + k1],
                        lamt_t[:, m * K + k0: m * K + k1],
                        P[:, 0:kw], op=mult)

        def pass_c(img):
            # WT[k-blk m, c-half hf] += Zp(l,m)^T @ Csel[l, hf]
            # boxskip: k-block m in the low strip only pairs with low l-chunks
            for i, (m, hf) in enumerate([(m, hf) for m in range(KB)
                                         for hf in range(2)]):
                if boxskip:
                    ls = range(0, NL) if m < NL else range(NL, KB)
                else:
                    ls = range(KB)
                ls = list(ls)
                P = pp.tile([128, 512], f32, name=f"C{img}_{m}_{hf}", tag="ps")
                for j, l in enumerate(ls):
                    nc.tensor.matmul(
                        P[:],
                        ZP[:, l * K + m * 128: l * K + (m + 1) * 128],
                        csel_t[:, l * NX + hf * 512: l * NX + hf * 512 + 512],
                        start=(j == 0), stop=(j == len(ls) - 1))
                evict(WT[:, m * NX + hf * 512: m * NX + hf * 512 + 512],
                      P[:], i)

        def pass_d(img):
            # XN[og g, c-half hf] += Csel(m,g)^T @ WT[m, hf]; overwrites X
            # (consumed by pass A); row-half DMA out on alternating engines
            X = Xs[img]
            for g in range(NB):
                Ph = []
                for hf in range(2):
                    P = pp.tile([128, 512], f32, name=f"D{img}_{g}_{hf}",
                                tag="ps")
                    Ph.append(P)
                    for m in range(KB):
                        nc.tensor.matmul(
                            P[:],
                            csel_t[:, m * NX + g * 128: m * NX + (g + 1) * 128],
                            WT[:, m * NX + hf * 512: m * NX + hf * 512 + 512],
                            start=(m == 0), stop=(m == KB - 1))
                for hf in range(2):
                    evict(X[:, g * NX + hf * 512: g * NX + hf * 512 + 512],
                          Ph[hf][:], g + hf)
                r0 = img * NX + g * 128
                src = X[:, g * NX:(g + 1) * NX].bitcast(f32)
                nc.sync.dma_start(y_d[r0:r0 + 64, :], src[0:64, :])
                nc.scalar.dma_start(y_d[r0 + 64:r0 + 128, :], src[64:128, :])

        pass_a(0)
        # remaining loads: dispatched while pass A streams, data arrives
        # during passes A/B (csel/lamt) and well before img1's pass A (X)
        for m in range(KB):
            nc.sync.dma_start(csel_t[:, m * NX:(m + 1) * NX], csel_d[m])
        for m in range(KB):
            nc.scalar.dma_start(lamt_t[:, m * K:(m + 1) * K], lamt_d[m])
        for b in range(NB):
            nc.sync.dma_start(Xs[1][:, b * NX:(b + 1) * NX],
                              x0_d[NX + b * 128: NX + (b + 1) * 128, :])
        pass_b(0)
        pass_c(0)
        pass_d(0)
        pass_a(1)
        pass_b(1)
        pass_c(1)
        pass_d(1)

    nc.compile()
    return nc, ctw_np, csel_np, lamt_np


def _host_reference(heat, n_iter):
    x = heat.reshape(16, NX, NX).astype(np.float32).copy()
    xp = np.empty((16, NX + 2, NX + 2), np.float32)
    for _ in range(n_iter):
        xp[:, 1:-1, 1:-1] = x
        xp[:, 0, 1:-1] = x[:, 1]
        xp[:, -1, 1:-1] = x[:, -2]
        xp[:, 1:-1, 0] = x[:, :, 1]
        xp[:, 1:-1, -1] = x[:, :, -2]
        x = 0.25 * (xp[:, :-2, 1:-1] + xp[:, 2:, 1:-1]
                    + xp[:, 1:-1, :-2] + xp[:, 1:-1, 2:])
    return x


def kernel(layout, heat, n_iter):
    n_iter = int(n_iter)
    heat = np.asarray(heat, dtype=np.float32)
    out_shape = heat.shape
    if n_iter <= 0:
        return heat.copy()

    nk = _pick_nk(n_iter)
    if nk is None:
        # n too small for safe spectral truncation (never hit for n=50);
        # exact host iteration keeps the kernel correct for any n_iter.
        return _host_reference(heat, n_iter).reshape(out_shape)

    from concourse.bass_utils import run_bass_kernel_spmd

    key = (n_iter, nk)
    if key not in _compiled_cache:
        _compiled_cache[key] = _build_program(n_iter, nk)
    nc, ctw_np, csel_np, lamt_np = _compiled_cache[key]

    x = heat.reshape(16, NX, NX)
    in_maps = []
    for c in range(N_CORES):
        shard = np.ascontiguousarray(
            x[c * IMGS_PER_CORE:(c + 1) * IMGS_PER_CORE].reshape(
                IMGS_PER_CORE * NX, NX))
        in_maps.append({"x0": shard, "ctw": ctw_np, "csel": csel_np,
                        "lamt": lamt_np})
    res = run_bass_kernel_spmd(nc, in_maps, core_ids=list(range(N_CORES)))
    out = np.empty((16, NX, NX), np.float32)
    for c in range(N_CORES):
        out[c * IMGS_PER_CORE:(c + 1) * IMGS_PER_CORE] = (
            res.results[c]["y"].reshape(IMGS_PER_CORE, NX, NX))
    return out.reshape(out_shape)


# revision 21
# speedup vs baseline: 16.6731x; 1.4211x over previous
"""Trainium2 Bass kernel for n-iteration Jacobi (3x3 cross stencil, reflect pad).

Spectral method: the iteration x <- 0.25*(V+H)x (+ f, dropped: |f| < 2.4e-9
contributes < 3e-7 relative) is exactly diagonalized by the DCT-I basis
v_k[i] = cos(pi*k*i/M), M = NX-1, eigenvalues lam_kl = (cos(pi k/M)+cos(pi l/M))/2.

  x_n = C^T [ Lam^n o (Cw x Cw^T) ] C        (o = elementwise)

lam^n kills all but the lowest and highest (checkerboard) frequency strips:
only K = 512 of 1024 modes per axis are kept (truncation err ~2e-4 for n=50,
verified on host), and of the KxK coefficient block only the (low,low) and
(high,high) boxes survive (cross boxes have |lam| <= 0.15 -> lam^50 ~ 1e-43).
Four dense matmul passes per image, ZERO transposes: alternate which operand
is stationary (the data X / Z' go in as lhsT).

  A: YT[c,k] = sum_i  X[i,c]   * CTw[i,k]     (lhsT = X blocks)
  B: Z [l,k] = sum_c  CTw[c,l] * YT[c,k]      (lhsT = CTw)   -> DVE *lam table
  C: WT[k,c] = sum_l  Zp[l,k]  * Csel[l,c]    (lhsT = Zp)
  D: XN[i,c] = sum_k  Csel[k,i]* WT[k,c]      (lhsT = Csel)

Passes A and D are folded by the mirror symmetry C[k, M-i] = (-1)^k C[k, i].
Kept modes are ordered [L-even | H-even | L-odd | H-odd] so each parity is a
contiguous half of the k axis:
  A: the host sends E = x[i]+x[M-i], O = x[i]-x[M-i] (i < 512); even-k
     coefficients contract E, odd-k contract O -> half the matmul columns.
  D: Se (even-k chunks 0,1) and So (odd, 2,3) are computed for rows i<512;
     XN[i] = Se+So, XN[M-i] = Se-So (written unreversed; the host reverses
     rows 512..1023 when unsharding) -> half the matmul columns.
Under this ordering the surviving (L,L) box is k cols {0:128}u{256:384},
handled with stride-2 block access patterns in pass B.

All matmul operands are fp16 (PSUM accumulates fp32); truncation + fp16
rounding measured 6e-4 relative on host, ~30x under the 2e-2 gate.  Input,
weights, and output stream as fp16, halving DMA volume; the host casts the
fp16 result back to fp32.

Per core: 2 of 16 images (data parallel over batch), everything SBUF-resident.
DMA dispatch is spread over the two HWDGE engines (sync + scalar); pass-A
inputs (ctw + EO img0) are dispatched first so matmuls start right after the
preamble; remaining loads are deferred behind pass A's emission.  gpsimd
softdge is a single slow queue (2.2us per 512KB) -- avoid it.
"""

import numpy as np
from contextlib import ExitStack

NX = 1024
M = NX - 1
NB = 8           # 128-row blocks per image
IMGS_PER_CORE = 2
N_CORES = 8

_compiled_cache = {}


def _pick_nk(n_iter):
    """Smallest strip width nk (K=2*nk kept modes/axis) with safe truncation.

    Dropped-mode field pointwise std ~ sqrt(sum_dropped lam^(2n)/(n_k n_l));
    require 6 sigma < 4e-4 (abs, vs |x|_max ~ 0.5 -> ~1e-3 relative budget).
    """
    lam1 = 0.5 * np.cos(np.pi * np.arange(NX) / M)
    nrm = np.full(NX, M / 2.0)
    nrm[0] = nrm[-1] = float(M)
    lam2 = np.abs(lam1[:, None] + lam1[None, :]) ** (2 * n_iter) / np.outer(nrm, nrm)
    for nk in (192, 256, 320, 384):
        keepmask = np.zeros(NX, bool)
        keepmask[:nk] = True
        keepmask[NX - nk:] = True
        drop = lam2 * ~(keepmask[:, None] & keepmask[None, :])
        if 6.0 * np.sqrt(drop.sum()) < 4e-4:
            return nk
    return None  # n too small for truncation with K<=768 -> host fallback


def _keep_order(nk, fold):
    low = np.arange(nk)
    high = np.arange(NX - nk, NX)
    if not fold:
        return np.r_[low, high]
    return np.r_[low[low % 2 == 0], high[high % 2 == 0],
                 low[low % 2 == 1], high[high % 2 == 1]]


def _build_tables(n_iter, nk, fold):
    keep = _keep_order(nk, fold)
    K = keep.size
    KB = K // 128
    idx = np.arange(NX)
    C = np.cos(np.pi * np.outer(keep, idx) / M)        # [K, NX]
    w = np.ones(NX)
    w[0] = w[-1] = 0.5
    nrm = np.full(NX, M / 2.0)
    nrm[0] = nrm[-1] = float(M)
    lam = 0.5 * np.cos(np.pi * keep / M)
    lam2 = (lam[:, None] + lam[None, :]) ** n_iter / np.outer(nrm[keep], nrm[keep])
    CTw = (C * w[None, :]).T                           # [NX, K]
    if fold:
        # folded analysis only needs rows i < 512
        ctw_np = np.ascontiguousarray(
            CTw[0:NX // 2].reshape(NB // 2, 128, K)).astype(np.float16)
    else:
        ctw_np = np.ascontiguousarray(
            CTw.reshape(NB, 128, K)).astype(np.float16)
    csel_np = np.ascontiguousarray(C.reshape(KB, 128, NX)).astype(np.float16)
    lamt_np = np.ascontiguousarray(lam2.reshape(KB, 128, K)).astype(np.float32)
    return ctw_np, csel_np, lamt_np


def _build_program(n_iter, nk):
    import concourse.bacc as bacc
    import concourse.mybir as mybir
    import concourse.tile as tile

    K = 2 * nk
    KB = K // 128
    KH = K // 2              # half-width of the kept-mode strip
    NL = KB // 2
    boxskip = (KB % 2 == 0) and (KH >= 256)
    fold = (nk % 256 == 0) and KB == 4
    ctw_np, csel_np, lamt_np = _build_tables(n_iter, nk, fold)
    NCH = ctw_np.shape[0]    # contraction chunks held in ctw (4 folded, 8 not)
    f16 = mybir.dt.float16
    f32 = mybir.dt.float32
    mult = mybir.AluOpType.mult
    add = mybir.AluOpType.add
    sub = mybir.AluOpType.subtract

    nc = bacc.Bacc("TRN2", target_bir_lowering=False, debug=False)
    x0_d = nc.dram_tensor("x0", [IMGS_PER_CORE * NX, NX], f16,
                          kind="ExternalInput").ap()
    ctw_d = nc.dram_tensor("ctw", [NCH, 128, K], f16,
                           kind="ExternalInput").ap()
    csel_d = nc.dram_tensor("csel", [KB, 128, NX], f16,
                            kind="ExternalInput").ap()
    lamt_d = nc.dram_tensor("lamt", [KB, 128, K], f32,
                            kind="ExternalInput").ap()
    y_d = nc.dram_tensor("y", [IMGS_PER_CORE * NX, NX], f16,
                         kind="ExternalOutput").ap()

    with tile.TileContext(nc) as tc, ExitStack() as ctx:
        wp = ctx.enter_context(tc.tile_pool(name="w", bufs=1))
        bp = ctx.enter_context(tc.tile_pool(name="b", bufs=1))
        pp = ctx.enter_context(tc.tile_pool(name="ps", bufs=8, space="PSUM"))

        ctw_t = wp.tile([128, NCH * K], f16, name="ctw", tag="ctw")
        csel_t = wp.tile([128, KB * NX], f16, name="csel", tag="csel")
        lamt_t = wp.tile([128, KB * K], f32, name="lamt", tag="lamt")

        Xs = [bp.tile([128, NB * NX], f16, name=f"x{i}", tag=f"x{i}")
              for i in range(IMGS_PER_CORE)]
        YT = bp.tile([128, NB * K], f16, name="yt", tag="yt")
        ZP = bp.tile([128, KB * K], f16, name="zp", tag="zp")
        WT = bp.tile([128, KB * NX], f16, name="wt", tag="wt")
        OUT = bp.tile([128, NB * NX], f16, name="out", tag="out")
        sp = ctx.enter_context(tc.tile_pool(name="s", bufs=4))

        # pass-A-critical loads first: ctw on scalar, X img0 on sync
        for b in range(NB):
            if b < NCH:
                nc.scalar.dma_start(ctw_t[:, b * K:(b + 1) * K], ctw_d[b])
            nc.sync.dma_start(Xs[0][:, b * NX:(b + 1) * NX],
                              x0_d[b * 128:(b + 1) * 128, :])

        def evict(dst_ap, src_ap, idx):
            if idx % 2 == 0:
                nc.scalar.copy(dst_ap, src_ap)
            else:
                nc.vector.tensor_copy(dst_ap, src_ap)

        def q2(ap, q0):
            # stride-2 view of 128-col blocks: cols {q0*128 + 0:128, +256:384}
            return ap.rearrange("p (q c) -> p q c", c=128)[:, q0::2, :]

        def pass_a_folded(img):
            # doubly-folded: X holds quadrants Q_pq at [p*512+i, q*512+c];
            # tile (g,q) accumulates parity-p halves over 4 chunks each,
            # p-major + chunk-major so X blocks stream in order 0..7
            X = Xs[img]
            wave = [(g, q) for g in range(4) for q in range(2)]
            P = {t: pp.tile([128, 512], f32, name=f"A{img}_{t[0]}_{t[1]}",
                            tag="ps") for t in wave}
            for p in range(2):
                for b in range(NCH):
                    rhs = ctw_t[:, b * K + p * 256: b * K + (p + 1) * 256]
                    for (g, q) in wave:
                        nc.tensor.matmul(
                            P[(g, q)][:, p * 256:(p + 1) * 256],
                            X[:, (p * NCH + b) * NX + q * 512 + g * 128:
                              (p * NCH + b) * NX + q * 512 + (g + 1) * 128],
                            rhs, start=(b == 0), stop=(b == NCH - 1))
            for i, (g, q) in enumerate(wave):
                evict(YT[:, (g * 2 + q) * 512:(g * 2 + q) * 512 + 512],
                      P[(g, q)][:], i)

        def pass_a_plain(img):
            X = Xs[img]
            for w0 in range(0, NB, 8):
                wave = list(range(w0, min(w0 + 8, NB)))
                P = {g: pp.tile([128, 512], f32, name=f"A{img}_{g}", tag="ps")
                     for g in wave}
                for b in range(NB):
                    for g in wave:
                        nc.tensor.matmul(
                            P[g][:, 0:min(K, 512)],
                            X[:, b * NX + g * 128: b * NX + (g + 1) * 128],
                            ctw_t[:, b * K: b * K + min(K, 512)],
                            start=(b == 0), stop=(b == NB - 1))
                for i, g in enumerate(wave):
                    evict(YT[:, g * K: g * K + min(K, 512)],
                          P[g][:, 0:min(K, 512)], i)

        pass_a = pass_a_folded if fold else pass_a_plain

        def pass_b(img):
            # Z[l-blk m, k-cols] += CTw(b,m)^T @ YT[b, k-cols]; DVE *lam evict.
            # folded: contract c<512 (4 chunks) against the q(m)-variant of
            # YT; only the in-box piece (type t) of each parity-p k-half.
            for m in range(KB):
                P = pp.tile([128, 512], f32, name=f"B{img}_{m}", tag="ps")
                if fold:
                    qm = 0 if m < 2 else 1
                    t = 0 if m % 2 == 0 else 1
                    for p in range(2):
                        for b in range(NCH):
                            nc.tensor.matmul(
                                P[:, p * 128:(p + 1) * 128],
                                ctw_t[:, b * K + m * 128:
                                      b * K + (m + 1) * 128],
                                YT[:, (b * 2 + qm) * 512 + p * 256 + t * 128:
                                   (b * 2 + qm) * 512 + p * 256 + t * 128
                                   + 128],
                                start=(b == 0), stop=(b == NCH - 1))
                    nc.vector.tensor_tensor(
                        q2(ZP[:, m * K:(m + 1) * K], t),
                        q2(lamt_t[:, m * K:(m + 1) * K], t),
                        P[:, 0:256].rearrange("p (q c) -> p q c", c=128),
                        op=mult)
                elif boxskip:
                    k0, k1 = (0, KH) if m < NL else (KH, K)
                    for b in range(NB):
                        nc.tensor.matmul(
                            P[:, 0:k1 - k0],
                            ctw_t[:, b * K + m * 128: b * K + (m + 1) * 128],
                            YT[:, b * K + k0: b * K + k1],
                            start=(b == 0), stop=(b == NB - 1))
                    nc.vector.tensor_tensor(
                        ZP[:, m * K + k0: m * K + k1],
                        lamt_t[:, m * K + k0: m * K + k1],
                        P[:, 0:k1 - k0], op=mult)
                else:
                    kw = min(K, 512)
                    for b in range(NB):
                        nc.tensor.matmul(
                            P[:, 0:kw],
                            ctw_t[:, b * K + m * 128: b * K + (m + 1) * 128],
                            YT[:, b * K: b * K + kw],
                            start=(b == 0), stop=(b == NB - 1))
                    nc.vector.tensor_tensor(
                        ZP[:, m * K: m * K + kw],
                        lamt_t[:, m * K: m * K + kw],
                        P[:, 0:kw], op=mult)

        def pass_c(img):
            # WT[k-blk m, c-half hf] += Zp(l,m)^T @ Csel[l, hf]
            for (m, hf) in [(m, hf) for m in range(KB) for hf in range(2)]:
                if fold:
                    ls = [0, 2] if m % 2 == 0 else [1, 3]
                elif boxskip:
                    ls = list(range(0, NL) if m < NL else range(NL, KB))
                else:
                    ls = list(range(KB))
                P = pp.tile([128, 512], f32, name=f"C{img}_{m}_{hf}", tag="ps")
                for j, l in enumerate(ls):
                    nc.tensor.matmul(
                        P[:],
                        ZP[:, l * K + m * 128: l * K + (m + 1) * 128],
                        csel_t[:, l * NX + hf * 512: l * NX + hf * 512 + 512],
                        start=(j == 0), stop=(j == len(ls) - 1))
                nc.scalar.copy(WT[:, m * NX + hf * 512: m * NX + hf * 512 + 512],
                               P[:])

        def pass_d_folded(img, last):
            # Se/So over rows i<512 only; XN[i]=Se+So, XN[M-i]=Se-So (host
            # reverses the second half).  Even-parity k chunks = blocks 0,1.
            ev, od = (0, 1), (2, 3)
            for g in range(4):
                Ps = {}
                for hf in range(2):
                    for par, chunks in (("e", ev), ("o", od)):
                        P = pp.tile([128, 512], f32,
                                    name=f"D{img}_{g}_{hf}{par}", tag="ps")
                        Ps[(hf, par)] = P
                        for j, m in enumerate(chunks):
                            nc.tensor.matmul(
                                P[:],
                                csel_t[:, m * NX + g * 128:
                                       m * NX + (g + 1) * 128],
                                WT[:, m * NX + hf * 512:
                                   m * NX + hf * 512 + 512],
                                start=(j == 0), stop=(j == len(chunks) - 1))
                for hf in range(2):
                    pe, po = Ps[(hf, "e")], Ps[(hf, "o")]
                    # DVE may read only ONE PSUM operand per op: stage Po in
                    # SBUF via ACT, then both +/- combines read Pe from PSUM
                    po_sb = sp.tile([128, 512], f32, name=f"po{img}_{g}_{hf}",
                                    tag="po")
                    nc.scalar.copy(po_sb[:], po[:])
                    nc.vector.tensor_tensor(
                        OUT[:, g * NX + hf * 512: g * NX + hf * 512 + 512],
                        pe[:], po_sb[:], op=add)
                    nc.vector.tensor_tensor(
                        OUT[:, (4 + g) * NX + hf * 512:
                            (4 + g) * NX + hf * 512 + 512],
                        pe[:], po_sb[:], op=sub)
                for half, blk in ((0, g), (1, 4 + g)):
                    r0 = img * NX + half * 512 + g * 128
                    src = OUT[:, blk * NX:(blk + 1) * NX]
                    if last and g >= 2:
                        nc.sync.dma_start(y_d[r0:r0 + 64, :], src[0:64, :])
                        nc.scalar.dma_start(y_d[r0 + 64:r0 + 128, :],
                                            src[64:128, :])
                    else:
                        eng = nc.sync if (g + half) % 2 == 0 else nc.scalar
                        eng.dma_start(y_d[r0:r0 + 128, :], src)

        def pass_d_plain(img, last):
            for g in range(NB):
                Ph = []
                for hf in range(2):
                    P = pp.tile([128, 512], f32, name=f"D{img}_{g}_{hf}",
                                tag="ps")
                    Ph.append(P)
                    for m in range(KB):
                        nc.tensor.matmul(
                            P[:],
                            csel_t[:, m * NX + g * 128: m * NX + (g + 1) * 128],
                            WT[:, m * NX + hf * 512: m * NX + hf * 512 + 512],
                            start=(m == 0), stop=(m == KB - 1))
                for hf in range(2):
                    evict(OUT[:, g * NX + hf * 512: g * NX + hf * 512 + 512],
                          Ph[hf][:], g + hf)
                r0 = img * NX + g * 128
                src = OUT[:, g * NX:(g + 1) * NX]
                if last and g >= NB - 2:
                    nc.sync.dma_start(y_d[r0:r0 + 64, :], src[0:64, :])
                    nc.scalar.dma_start(y_d[r0 + 64:r0 + 128, :], src[64:128, :])
                else:
                    eng = nc.sync if g % 2 == 0 else nc.scalar
                    eng.dma_start(y_d[r0:r0 + 128, :], src)

        pass_d = pass_d_folded if fold else pass_d_plain

        pass_a(0)
        # remaining loads: dispatched while pass A streams
        nh = KB // 2
        for h in range(2):
            nc.sync.dma_start(
                csel_t[:, h * nh * NX:(h + 1) * nh * NX].rearrange(
                    "p (b c) -> p b c", c=NX),
                csel_d[h * nh:(h + 1) * nh].rearrange("b p c -> p b c"))
        nc.scalar.dma_start(
            lamt_t[:].rearrange("p (b c) -> p b c", c=K),
            lamt_d[:].rearrange("b p c -> p b c"))
        for b in range(NB):
            nc.sync.dma_start(Xs[1][:, b * NX:(b + 1) * NX],
                              x0_d[NX + b * 128: NX + (b + 1) * 128, :])
        pass_b(0)
        pass_c(0)
        pass_d(0, last=False)
        pass_a(1)
        pass_b(1)
        pass_c(1)
        pass_d(1, last=True)

    nc.compile()
    return nc, ctw_np, csel_np, lamt_np, fold


def _host_reference(heat, n_iter):
    x = heat.reshape(16, NX, NX).astype(np.float32).copy()
    xp = np.empty((16, NX + 2, NX + 2), np.float32)
    for _ in range(n_iter):
        xp[:, 1:-1, 1:-1] = x
        xp[:, 0, 1:-1] = x[:, 1]
        xp[:, -1, 1:-1] = x[:, -2]
        xp[:, 1:-1, 0] = x[:, :, 1]
        xp[:, 1:-1, -1] = x[:, :, -2]
        x = 0.25 * (xp[:, :-2, 1:-1] + xp[:, 2:, 1:-1]
                    + xp[:, 1:-1, :-2] + xp[:, 1:-1, 2:])
    return x


def _make_shard(ximgs, fold):
    """[imgs, NX, NX] f32 -> [imgs*NX, NX] fp16 device layout.

    fold: quadrants Q_pq[i,c] = sum of the four mirror images with signs
    (-1)^(p s) (-1)^(q t), stored at [p*512 + i, q*512 + c].
    """
    if not fold:
        return np.ascontiguousarray(
            ximgs.astype(np.float16).reshape(-1, NX))
    h = NX // 2
    a = ximgs[:, 0:h, 0:h]
    ar = ximgs[:, ::-1, :][:, 0:h, 0:h]
    ac = ximgs[:, :, ::-1][:, 0:h, 0:h]
    arc = ximgs[:, ::-1, ::-1][:, 0:h, 0:h]
    out = np.empty((ximgs.shape[0], NX, NX), np.float16)
    for p in (0, 1):
        sp_ = -1.0 if p else 1.0
        for q in (0, 1):
            sq = -1.0 if q else 1.0
            out[:, p * h:(p + 1) * h, q * h:(q + 1) * h] = (
                a + sp_ * ar + sq * ac + sp_ * sq * arc)
    return np.ascontiguousarray(out.reshape(-1, NX))


def kernel(layout, heat, n_iter):
    n_iter = int(n_iter)
    heat = np.asarray(heat, dtype=np.float32)
    out_shape = heat.shape
    if n_iter <= 0:
        return heat.copy()

    nk = _pick_nk(n_iter)
    if nk is None:
        # n too small for safe spectral truncation (never hit for n=50);
        # exact host iteration keeps the kernel correct for any n_iter.
        return _host_reference(heat, n_iter).reshape(out_shape)

    from concourse.bass_utils import run_bass_kernel_spmd

    key = (n_iter, nk)
    if key not in _compiled_cache:
        _compiled_cache[key] = _build_program(n_iter, nk)
    nc, ctw_np, csel_np, lamt_np, fold = _compiled_cache[key]

    x = heat.reshape(16, NX, NX)
    in_maps = []
    for c in range(N_CORES):
        shard = _make_shard(x[c * IMGS_PER_CORE:(c + 1) * IMGS_PER_CORE], fold)
        in_maps.append({"x0": shard, "ctw": ctw_np, "csel": csel_np,
                        "lamt": lamt_np})
    res = run_bass_kernel_spmd(nc, in_maps, core_ids=list(range(N_CORES)))
    out = np.empty((16, NX, NX), np.float32)
    for c in range(N_CORES):
        y = res.results[c]["y"].reshape(IMGS_PER_CORE, NX, NX)
        if fold:
            y = y.copy()
            y[:, NX // 2:] = y[:, NX // 2:][:, ::-1]
        out[c * IMGS_PER_CORE:(c + 1) * IMGS_PER_CORE] = y
    return out.reshape(out_shape)
